# revision 6
# baseline (speedup 1.0000x reference)
"""Llama4 MoE experts kernel, int8-W1 variant.

Same math/sharding as kernel.py (bf16 variant), but W1 (gate_up_proj) is
stored in HBM as int8 (host-quantized, 4-sigma clip, per-expert scale) and
expanded to bf16 by the gpsimd *casting DMA* on the way into SBUF. W2
stays bf16: the binding limit is the SBUF AXI write side (~100 MB bf16 /
435 GB/s ~ 231 us), which int8 storage does not change, so quantizing W2
would add error (1.7% vs 1.4% total) for zero speed. HBM reads drop to
~67 MB/core (187 us) — comfortably under the SBUF-write floor.

Scale folding keeps the dequant a pure int8->bf16 cast (no multiplies on
the wide path):
  - matmul1 psums are in "int units" of W1; silu gets scale=s1 via the
    activation's pre-scale: sil = silu(s1 * ps_gate)  (exact, per token)
  - h = sil * ps_up carries a pending factor s1
  - matmul2 (real-valued W2) output carries s1, applied by the output
    copy's activation scale.
Scales arrive as a tiny runtime input tensor [EPC, T, 2] (s1 in both
columns, replicated across tokens to serve as per-partition scale APs).

v4: weights are additionally *host-repacked into SBUF-tile order* so each
DMA chunk is per-partition contiguous in HBM. That turns each descriptor
from one (partition, k-row) 2-4 KiB segment into one 16-32 KiB
per-partition segment (8x fewer descriptors). Measured v3/bf16 ran at
~205 ns per 4 KiB SBUF-write descriptor (~320 GB/s effective, descriptor-
rate bound); 32 KiB descriptors track the ~400+ GB/s curve.
"""

import contextlib

import numpy as np

import concourse.bass as bass
import concourse.mybir as mybir
import concourse.tile as tile
from concourse import bacc
from concourse.bass import ds
from concourse.bass_utils import run_bass_kernel_spmd
from concourse.masks import make_identity

E, T, H, I = 16, 64, 2048, 4096
NCORES = 8
EPC = E // NCORES
P = 128
NT = 512
WT = 2048          # weight-DMA column tile (2 KiB int8 contiguous per descriptor)
KC = 8
KSUB1 = H // P
KSUB2 = I // P
NJ1 = 2 * I // WT  # 4 interleaved gate|up groups
GW = 1024

F32 = mybir.dt.float32
BF16 = mybir.dt.bfloat16
I8 = mybir.dt.int8
CLIP_SIGMA = 4.0


def build_program(repeat: int = 1) -> bass.Bass:
    nc = bacc.Bacc(None, target_bir_lowering=False, debug=False)

    hidden = nc.dram_tensor("hidden_states", [EPC, T, H], BF16, kind="ExternalInput")
    # Weights are host-repacked into SBUF-tile order:
    #   w1[e, j*2+kc, p, k*WT+w] = w1_interleaved[e, (kc*KC+k)*P+p, j*WT+w]
    #   w2[e, kc, p, k*H+w]      = w2[e, (kc*KC+k)*P+p, w]
    w1 = nc.dram_tensor(
        "gate_up_proj", [EPC, 2 * NJ1, P, KC * WT], I8, kind="ExternalInput"
    )
    w2 = nc.dram_tensor(
        "down_proj", [EPC, KSUB2 // KC, P, KC * H], BF16, kind="ExternalInput"
    )
    scales = nc.dram_tensor("scales", [EPC, T, 2], F32, kind="ExternalInput")
    out = nc.dram_tensor("out", [EPC, T, H], BF16, kind="ExternalOutput")

    with tile.TileContext(nc) as tc:
        with (
            tc.tile_pool(name="const", bufs=1) as const,
            tc.tile_pool(name="wpool", bufs=4) as wpool,
            tc.tile_pool(name="xpool", bufs=2) as xpool,
            tc.tile_pool(name="scpool", bufs=2) as scpool,
            tc.tile_pool(name="xtpool", bufs=2) as xtpool,
            tc.tile_pool(name="htpool", bufs=2) as htpool,
            tc.tile_pool(name="spool", bufs=3) as spool,
            tc.tile_pool(name="opool", bufs=2) as opool,
            tc.tile_pool(name="mmps", bufs=6, space="PSUM") as mmps,
            tc.tile_pool(name="trps", bufs=2, space="PSUM") as trps,
        ):
            ident = const.tile([T, T], BF16, name="ident")
            make_identity(nc, ident)

            # Unroll the benchmark loop body x4: if For_i pays a cross-engine
            # sync at the back edge, amortize it over 4 iterations.
            UNROLL = 4
            if repeat > 1:
                assert repeat % UNROLL == 0, repeat
                loop_cm, n_body = tc.For_i(0, repeat // UNROLL, 1), UNROLL
            else:
                loop_cm, n_body = contextlib.nullcontext(), 1
            with loop_cm:
                for _ in range(n_body):
                    body(nc, hidden, w1, w2, scales, out, wpool, xpool, scpool,
                         xtpool, htpool, spool, opool, mmps, trps, ident)

    nc.compile()
    return nc


def body(nc, hidden, w1, w2, scales, out, wpool, xpool, scpool, xtpool,
         htpool, spool, opool, mmps, trps, ident):
    for e in range(EPC):
        x_sb = xpool.tile([T, H], BF16, name="x_sb", tag="x")
        nc.sync.dma_start(x_sb[:], hidden[e])
        sc = scpool.tile([T, 2], F32, name="sc", tag="sc")
        nc.sync.dma_start(sc[:], scales[e])

        xT = xtpool.tile([P, KSUB1, T], BF16, name="xT", tag="xT")
        for ko in range(KSUB1):
            tp = trps.tile([P, T], BF16, name="tp", tag="tp")
            nc.tensor.transpose(tp[:], x_sb[:, ds(ko * P, P)], ident[:])
            nc.vector.tensor_copy(xT[:, ko, :], tp[:])

        hT = htpool.tile([P, KSUB2, T], BF16, name="hT", tag="hT")

        # ---- matmul 1 + SwiGLU over interleaved [1024 gate | 1024 up] groups ----
        for j in range(NJ1):
            ps = [
                mmps.tile([T, NT], F32, name=f"ps{b}", tag="mm")
                for b in range(4)
            ]
            for kc in range(KSUB1 // KC):
                wt = wpool.tile([P, KC, WT], BF16, name="wt", tag="w")
                nc.gpsimd.dma_start(wt[:], w1[e, j * 2 + kc])
                for k in range(KC):
                    ko = kc * KC + k
                    for b in range(4):
                        nc.tensor.matmul(
                            ps[b][:],
                            xT[:, ko, :],
                            wt[:, k, ds(b * NT, NT)],
                            start=(ko == 0),
                            stop=(ko == KSUB1 - 1),
                        )
            for half in range(2):
                sil = spool.tile([T, NT], F32, name="sil", tag="sil")
                nc.scalar.activation(
                    sil[:], ps[half][:], mybir.ActivationFunctionType.Silu,
                    scale=sc[:, 0:1],
                )
                h_sb = spool.tile([T, NT], BF16, name="h_sb", tag="h")
                nc.vector.tensor_mul(h_sb[:], sil[:], ps[2 + half][:])

                for i in range(NT // P):
                    tp2 = trps.tile([P, T], BF16, name="tp2", tag="tp")
                    nc.tensor.transpose(tp2[:], h_sb[:, ds(i * P, P)], ident[:])
                    kidx = (GW // P) * j + (NT // P) * half + i
                    nc.vector.tensor_copy(hT[:, kidx, :], tp2[:])

        # ---- matmul 2 ----
        ops = [
            mmps.tile([T, NT], F32, name=f"ops{b}", tag="mm")
            for b in range(H // NT)
        ]
        for kc in range(KSUB2 // KC):
            wt2 = wpool.tile([P, KC, H], BF16, name="wt2", tag="w")
            eng = nc.sync if kc % 2 == 0 else nc.scalar
            eng.dma_start(wt2[:], w2[e, kc])
            for k in range(KC):
                ko = kc * KC + k
                for b in range(H // NT):
                    nc.tensor.matmul(
                        ops[b][:],
                        hT[:, ko, :],
                        wt2[:, k, ds(b * NT, NT)],
                        start=(ko == 0),
                        stop=(ko == KSUB2 - 1),
                    )
        o_sb = opool.tile([T, H], BF16, name="o_sb", tag="o")
        for b in range(H // NT):
            nc.scalar.activation(
                o_sb[:, ds(b * NT, NT)], ops[b][:],
                mybir.ActivationFunctionType.Copy, scale=sc[:, 1:2],
            )
        nc.scalar.dma_start(out[e], o_sb[:])


def _quantize(w: np.ndarray) -> tuple[np.ndarray, np.ndarray]:
    """Per-expert symmetric int8 quantization with 4-sigma clip.
    Returns (q [E,...] int8, step [E] f32)."""
    flat = w.reshape(w.shape[0], -1)
    sigma = flat.std(axis=1)
    step = CLIP_SIGMA * sigma / 127.0
    q = np.clip(
        np.round(w / step.reshape(-1, *([1] * (w.ndim - 1)))), -127, 127
    ).astype(np.int8)
    return q, step.astype(np.float32)


def preprocess(inputs: dict) -> list:
    import ml_dtypes

    bf16 = ml_dtypes.bfloat16

    hs = np.ascontiguousarray(
        np.asarray(inputs["hidden_states"], dtype=np.float32)
    ).astype(bf16)
    w1 = np.ascontiguousarray(np.asarray(inputs["gate_up_proj"], dtype=np.float32))
    w2 = np.ascontiguousarray(
        np.asarray(inputs["down_proj"], dtype=np.float32)
    ).astype(bf16)

    w1q, s1 = _quantize(w1)

    # Interleave W1 columns: [j*2048, +1024) = gate[j*1024, +1024),
    #                        [j*2048+1024, +1024) = up[j*1024, +1024).
    w1g = w1q[:, :, :I].reshape(E, H, I // GW, GW)
    w1u = w1q[:, :, I:].reshape(E, H, I // GW, GW)
    w1i = np.stack([w1g, w1u], axis=3).reshape(E, H, 2 * I)

    # Repack into SBUF-tile order (per-partition contiguous chunks):
    # H = (kc, k, p); chunk (j, kc) holds [p, k*WT+w] = w1i[(kc*KC+k)*P+p, j*WT+w]
    w1p = np.ascontiguousarray(
        w1i.reshape(E, KSUB1 // KC, KC, P, NJ1, WT)
        .transpose(0, 4, 1, 3, 2, 5)
        .reshape(E, 2 * NJ1, P, KC * WT)
    )
    # W2: I = (kc, k, p); chunk kc holds [p, k*H+w] = w2[(kc*KC+k)*P+p, w]
    w2p = np.ascontiguousarray(
        w2.reshape(E, KSUB2 // KC, KC, P, H)
        .transpose(0, 1, 3, 2, 4)
        .reshape(E, KSUB2 // KC, P, KC * H)
    )

    scales = np.zeros((E, T, 2), np.float32)
    scales[:, :, 0] = s1[:, None]
    scales[:, :, 1] = s1[:, None]

    in_maps = []
    for c in range(NCORES):
        sl = slice(c * EPC, (c + 1) * EPC)
        in_maps.append(
            {
                "hidden_states": np.ascontiguousarray(hs[sl]),
                "gate_up_proj": np.ascontiguousarray(w1p[sl]),
                "down_proj": np.ascontiguousarray(w2p[sl]),
                "scales": np.ascontiguousarray(scales[sl]),
            }
        )
    return in_maps


_NC_CACHE = None


def _get_program():
    global _NC_CACHE
    if _NC_CACHE is None:
        _NC_CACHE = build_program()
    return _NC_CACHE


def run(inputs: dict, trace: bool = False):
    in_maps = preprocess(inputs)
    nc = _get_program()
    res = run_bass_kernel_spmd(nc, in_maps, core_ids=list(range(NCORES)), trace=trace)
    out = np.concatenate(
        [np.asarray(r["out"], dtype=np.float32) for r in res.results], axis=0
    )
    return out, res


def kernel(**inputs) -> np.ndarray:
    out, _ = run(inputs, trace=False)
    return out
